# revision 44
# baseline (speedup 1.0000x reference)
"""GAT forward on 8 Trainium2 NeuronCores (Bass/Tile, SPMD, no collectives).

Sharding: edges assigned to cores by src-node range (N/8 nodes per core);
each core computes the full output rows for its own src range.

Structure (v4): a bf16 node table [N, 256] holds [xW | 1 | v' | u'] per
node (512B stride, b NOT folded in). Edge-tile rows are fetched with
InstDMAGatherAnt (260B payload per edge, int16 indices, so each src
block's edges are split into dst<32768 / dst>=32768 groups gathered from
base-offset views). Per-edge u'[src] is selected from a per-block u
column by a tiny PE matmul against a host-streamed fp8 transposed
one-hot; exp(leaky_relu) is computed on [128, tiles] columns with the
b/a_bias constants folded into the activation bias; the one-hot scatter
matrix is a single fused tensor_scalar(is_equal, mult) per tile feeding
a bf16 aggregation matmul with PSUM accumulation per block. The linear
bias b is added at the batched ELU finisher (sum(alpha)=1 makes this
exact). Softmax max-subtraction is skipped: logits are bounded so exp is
well-conditioned and the result is mathematically identical.

v4 vs v3: phase 1 drops the rank-1 bias matmul (consts -> exp bias,
b -> finisher add), batches 3 matmul outputs per PSUM bank with one
Scalar copy per group, and memsets the "1" column; per-block u-column
indirect gathers are spread through the block loop; pipeline pools are
deepened so SWDGE gathers run several halves ahead.
"""
import math

import numpy as np

N, E, D, P = 50000, 1600000, 128, 128
NCORES = 8
NPC = N // NCORES          # nodes per core
NBLK = math.ceil(NPC / P)  # src blocks per core
STEP = 256                 # table row stride (bf16 elems; 512B)
ELW = 131                  # table row payload written: [xW |1| v' | u']
ELG = 130                  # gathered payload elems: [xW |1| v'] (260B)
PAD_SRC = 999.0
SPLIT = 32768              # int16 index range split
IC = 256                   # tiles per srcT DRAM load
EC = 16                    # tiles per eqT DRAM load
GF = 4                     # blocks per batched ELU finisher
GMAX = 4                   # tiles per dma_gather (512 desc; 2 fit per ring)

_cache = {}


def _dma_gather(gp, out_ap, in_ap, idxs_ap, num_idxs, elem_size, elem_step,
                queue_num=0):
    """dma_gather (non-transpose, HBM source) with the %256 payload assert
    relaxed; the row stride must still be a 256B multiple (HW field)."""
    import concourse.mybir as mybir
    from concourse import ap_utils

    assert idxs_ap.dtype == mybir.dt.int16
    assert in_ap.dtype == out_ap.dtype
    dts = mybir.dt.size(in_ap.dtype)
    assert ap_utils.ap_is_contiguous(out_ap.ap[1:])
    assert ap_utils.ap_is_contiguous(idxs_ap.ap[1:])
    assert num_idxs % 128 == 0
    assert in_ap.ap[-1][1] == elem_size and out_ap.ap[-1][1] == elem_size
    assert out_ap.ap[0][1] * out_ap.ap[1][1] == num_idxs
    stride_bytes = elem_step * dts
    assert stride_bytes % 256 == 0 and stride_bytes // 256 < 256
    assert in_ap.ap[0][0] == elem_step
    _in_ap = gp.lower_ap_dma(in_ap, for_custom_bir_dma=True)
    _idxs_ap = gp.lower_ap(idxs_ap)
    _out_ap = gp.lower_ap(out_ap)
    return gp.add_instruction(
        mybir.InstDMAGatherAnt(
            name=gp.bass.get_next_instruction_name(),
            ins=[*_in_ap, _idxs_ap, gp.lower_val_access(gp.to_reg(num_idxs))],
            outs=[_out_ap],
            transpose=False,
            num_idxs=num_idxs,
            elem_size=elem_size,
            stride_bytes_256=stride_bytes // 256,
            gen_mode=0,
            single_packet=True,
            queue_num=queue_num,
            sbuf_tokens_per_rank=0,
            sbuf_free_dim_per_rank=0,
            sbuf_free_dim_pad_per_rank=0,
            sbuf_byte_offset=0,
        ))


def _build_program(TL, TH):
    """Build the SPMD bass program. TL/TH = per-block tile counts for the
    low/high dst halves (len NBLK each, shared schedule across cores)."""
    from contextlib import ExitStack
    import concourse.bass as bass
    import concourse.bacc as bacc
    import concourse.mybir as mybir
    import concourse.tile as tile

    T_per = [int(TL[b]) + int(TH[b]) for b in range(NBLK)]
    T_total = int(sum(T_per))
    Tmax = int(max(max(TL), max(TH)))
    n_nt = math.ceil(N / P)            # phase-1 node tiles
    N_pad = n_nt * P
    nc = bacc.Bacc("TRN2", target_bir_lowering=False, debug=False,
                   num_swdge_queues=4)

    bf16 = mybir.dt.bfloat16
    f32 = mybir.dt.float32
    i32 = mybir.dt.int32
    i16 = mybir.dt.int16
    f8 = mybir.dt.float8e4

    xT = nc.dram_tensor("xT", [P, N_pad], bf16, kind="ExternalInput")
    Wp = nc.dram_tensor("Wp", [P, ELW], bf16, kind="ExternalInput")
    cb = nc.dram_tensor("cb", [1, 2 + GF * P], f32, kind="ExternalInput")
    idxW = nc.dram_tensor("idxW", [P, T_total * 8], i16, kind="ExternalInput")
    srcT = nc.dram_tensor("srcT", [P, T_total], f32, kind="ExternalInput")
    eqT8 = nc.dram_tensor("eqT8", [P, T_total * P], f8, kind="ExternalInput")
    out = nc.dram_tensor("out", [NPC, D], f32, kind="ExternalOutput")

    with tile.TileContext(nc) as tc, ExitStack() as ctx:
        const_p = ctx.enter_context(tc.tile_pool(name="const", bufs=1))
        dram_p = ctx.enter_context(tc.tile_pool(name="dram", bufs=1, space="DRAM"))
        x_p = ctx.enter_context(tc.tile_pool(name="x", bufs=2))
        h_p = ctx.enter_context(tc.tile_pool(name="h", bufs=3))
        ps1_p = ctx.enter_context(tc.tile_pool(name="ps1", bufs=4, space="PSUM"))
        idx_p = ctx.enter_context(tc.tile_pool(name="idx", bufs=8))
        eq_p = ctx.enter_context(tc.tile_pool(name="eq", bufs=5))
        g_p = ctx.enter_context(tc.tile_pool(name="g", bufs=8))
        sc_p = ctx.enter_context(tc.tile_pool(name="sc", bufs=4))
        ups_p = ctx.enter_context(tc.tile_pool(name="ups", bufs=2, space="PSUM"))
        mex_p = ctx.enter_context(tc.tile_pool(name="mex", bufs=8))
        acc_ps = ctx.enter_context(tc.tile_pool(name="accps", bufs=2, space="PSUM"))
        fin_p = ctx.enter_context(tc.tile_pool(name="fin", bufs=3))

        h_ext = dram_p.tile([N_pad, STEP], bf16)

        # constants
        iota_i = const_p.tile([P, P], i32)
        nc.gpsimd.iota(iota_i[:], [[1, P]], channel_multiplier=0)
        iota_b = const_p.tile([P, P], bf16)
        nc.vector.tensor_copy(iota_b[:], iota_i[:])
        ones_row = const_p.tile([1, P], bf16)
        nc.vector.memset(ones_row[:], 1.0)

        Wp_t = const_p.tile([P, ELW], bf16)
        nc.sync.dma_start(Wp_t[:], Wp[:, :])
        u_cols = const_p.tile([P, NBLK], bf16)
        # broadcast [1, 2+GF*P] consts (C, 0.01*C, b tiled GF times) to all
        # partitions via contraction-1 PE matmuls (PSUM bank caps at 512 f32)
        ones1f = const_p.tile([1, P], f32)
        nc.vector.memset(ones1f[:], 1.0)
        neg1_col = const_p.tile([P, 1], f32)
        nc.vector.memset(neg1_col[:], -1.0)
        cb_row = const_p.tile([1, 2 + GF * P], f32)
        nc.sync.dma_start(cb_row[:], cb[:, :])
        cb_t = const_p.tile([P, 2 + GF * P], f32)
        cbC_ps = ps1_p.tile([P, 2], f32, tag="ps1")
        nc.tensor.matmul(cbC_ps[:], lhsT=ones1f[:], rhs=cb_row[:, 0:2],
                         start=True, stop=True)
        nc.scalar.copy(cb_t[:, 0:2], cbC_ps[:])
        b4_ps = ps1_p.tile([P, GF * P], f32, tag="ps1")
        nc.tensor.matmul(b4_ps[:], lhsT=ones1f[:], rhs=cb_row[:, 2:2 + GF * P],
                         start=True, stop=True)
        nc.scalar.copy(cb_t[:, 2:2 + GF * P], b4_ps[:])
        C_col = cb_t[:, 0:1]
        C2_col = cb_t[:, 1:2]
        b_rep = cb_t[:, 2:2 + GF * P]

        # ---- phase 1: h_ext[n, 0:ELW] = [xW | 1 | v' | u'] (bf16) ----
        XC = 16  # node tiles per x chunk / per staged table write
        for c0 in range(0, n_nt, XC):
            cn = min(XC, n_nt - c0)
            xc = x_p.tile([P, XC * P], bf16, tag="xc")
            nc.sync.dma_start(xc[:, :cn * P], xT[:, c0 * P:(c0 + cn) * P])
            hs = h_p.tile([P, XC * ELW], bf16, tag="hs")
            for g0 in range(0, cn, 3):
                gn = min(3, cn - g0)
                ps = ps1_p.tile([P, 3 * ELW], f32, tag="ps1")
                for k in range(gn):
                    j = g0 + k
                    nc.tensor.matmul(ps[:, k * ELW:(k + 1) * ELW],
                                     lhsT=xc[:, j * P:(j + 1) * P],
                                     rhs=Wp_t[:], start=True, stop=True)
                nc.vector.tensor_copy(hs[:, g0 * ELW:(g0 + gn) * ELW],
                                      ps[:, :gn * ELW])
            hs_v = hs[:].rearrange("p (j c) -> p j c", c=ELW)
            nc.vector.memset(hs_v[:, :cn, 128:129], 1.0)
            # the table is rotated per-core so this core's src blocks are
            # tiles 0..NBLK-1: peel their u' columns off the staging tile
            for j in range(cn):
                gidx = c0 + j
                if gidx < NBLK:
                    nc.vector.tensor_copy(
                        u_cols[:, gidx:gidx + 1],
                        hs[:, j * ELW + 130:j * ELW + 131])
            dst = h_ext[c0 * P:(c0 + cn) * P, 0:ELW]
            nc.sync.dma_start(
                dst.rearrange("(j p) c -> p j c", p=P),
                hs[:, :cn * ELW].rearrange("p (j c) -> p j c", c=ELW))

        # ---- phase 2 ----
        src_tiles = {}

        def get_src(t):
            c0 = (t // IC) * IC
            if c0 not in src_tiles:
                cn = min(IC, T_total - c0)
                s_c = idx_p.tile([P, IC], f32, tag="srcc")
                nc.scalar.dma_start(s_c[:, :cn], srcT[:, c0:c0 + cn])
                src_tiles[c0] = s_c
            return src_tiles[c0], t - c0

        eq_tiles = {}

        def get_eq(t):
            c0 = (t // EC) * EC
            if c0 not in eq_tiles:
                cn = min(EC, T_total - c0)
                e_c = eq_p.tile([P, EC * P], f8, tag="eqc")
                nc.scalar.dma_start(e_c[:, :cn * P],
                                    eqT8[:, c0 * P:(c0 + cn) * P])
                eq_tiles[c0] = e_c
            return eq_tiles[c0], t - c0

        # batched ELU finisher staging
        stage = None
        stage_blocks = []

        def flush_stage():
            nonlocal stage, stage_blocks
            if not stage_blocks:
                return
            w = len(stage_blocks) * P
            yb = fin_p.tile([P, GF * P], f32, tag="yb")
            nc.vector.tensor_tensor(out=yb[:, :w], in0=stage[:, :w],
                                    in1=b_rep[:, :w], op=mybir.AluOpType.add)
            mp = fin_p.tile([P, GF * P], f32, tag="mp")
            nc.vector.tensor_scalar(
                out=mp[:, :w], in0=yb[:, :w], scalar1=0.0, scalar2=1.0,
                op0=mybir.AluOpType.min, op1=mybir.AluOpType.add)
            ex = fin_p.tile([P, GF * P], f32, tag="ex")
            nc.scalar.activation(ex[:, :w], mp[:, :w],
                                 mybir.ActivationFunctionType.Exp,
                                 bias=neg1_col[:])
            tm = fin_p.tile([P, GF * P], f32, tag="tm")
            nc.vector.tensor_tensor(out=tm[:, :w], in0=yb[:, :w],
                                    in1=mp[:, :w], op=mybir.AluOpType.subtract)
            ot = fin_p.tile([P, GF * P], f32, tag="ot")
            nc.vector.tensor_tensor(out=ot[:, :w], in0=tm[:, :w],
                                    in1=ex[:, :w], op=mybir.AluOpType.add)
            for gi, b in enumerate(stage_blocks):
                rows_b = min(P, NPC - b * P)
                nc.sync.dma_start(out[b * P:b * P + rows_b, :],
                                  ot[:rows_b, gi * P:gi * P + D])
            stage = None
            stage_blocks = []

        # flat half schedule: (block, th, base, t0, first, last, nhalf)
        sched = []
        t = 0
        for b in range(NBLK):
            nhalf = int(TL[b]) + int(TH[b])
            halves = [(th, base) for th, base in
                      ((int(TL[b]), 0), (int(TH[b]), SPLIT)) if th]
            for hi, (th, base) in enumerate(halves):
                sched.append((b, th, base, t, hi == 0,
                              hi == len(halves) - 1, nhalf))
                t += th

        next_q = [0]

        def stage_a(i):
            """Prologue for half i: idx load, gathers, u' select, raw."""
            b, th, base, t0, _, _, _ = sched[i]
            ix = idx_p.tile([P, Tmax * 8], i16, tag="ix")
            nc.scalar.dma_start(ix[:, :th * 8], idxW[:, t0 * 8:(t0 + th) * 8])
            g_c = g_p.tile([P, Tmax, ELG], bf16, tag="g")
            # ucode SWDGE ring caps 1024 descriptors; round-robin 4 queues
            for k0 in range(0, th, GMAX):
                gn = min(GMAX, th - k0)
                _dma_gather(nc.gpsimd, g_c[:, k0:k0 + gn, :],
                            h_ext[base:, 0:ELG],
                            ix[:, k0 * 8:(k0 + gn) * 8],
                            gn * P, ELG, STEP,
                            queue_num=next_q[0])
                next_q[0] = (next_q[0] + 1) % 4
            # u'[src_e] columns via fp8 one-hot matmuls
            u_ps = ups_p.tile([P, Tmax], f32, tag="ups")
            for j in range(th):
                e_c, j_loc = get_eq(t0 + j)
                nc.tensor.matmul(
                    u_ps[:, j:j + 1],
                    lhsT=e_c[:, j_loc * P:(j_loc + 1) * P],
                    rhs=u_cols[:, b:b + 1], start=True, stop=True)
            # s = u' + v';  raw = max(exp(s + C), exp(.01 (s + C)))
            s_c = sc_p.tile([P, Tmax], f32, tag="s")
            nc.vector.tensor_tensor(
                out=s_c[:, :th], in0=u_ps[:, :th],
                in1=g_c[:, :th, 129:130], op=mybir.AluOpType.add)
            e1 = sc_p.tile([P, Tmax], f32, tag="e1")
            nc.scalar.activation(e1[:, :th], s_c[:, :th],
                                 mybir.ActivationFunctionType.Exp,
                                 bias=C_col)
            e2 = sc_p.tile([P, Tmax], f32, tag="e2")
            nc.scalar.activation(e2[:, :th], s_c[:, :th],
                                 mybir.ActivationFunctionType.Exp,
                                 scale=0.01, bias=C2_col)
            raw = sc_p.tile([P, Tmax], f32, tag="raw")
            nc.vector.tensor_tensor(out=raw[:, :th], in0=e1[:, :th],
                                    in1=e2[:, :th], op=mybir.AluOpType.max)
            return g_c, raw

        acc_live = [None, 0]

        def stage_b(i, g_c, raw):
            """Tile loop for half i: one-hot scatter matmuls + finisher."""
            nonlocal stage, stage_blocks
            b, th, base, t0, first, last, nhalf = sched[i]
            if first:
                acc = acc_ps.tile([P, D + 1], f32, tag="acc")
                acc_live[0] = acc
                acc_live[1] = 0
            acc = acc_live[0]
            for j in range(th):
                s_t, s_loc = get_src(t0 + j)
                mex = mex_p.tile([P, P], bf16, tag="mex")
                nc.vector.tensor_scalar(
                    out=mex[:], in0=iota_b[:],
                    scalar1=s_t[:, s_loc:s_loc + 1],
                    scalar2=raw[:, j:j + 1],
                    op0=mybir.AluOpType.is_equal,
                    op1=mybir.AluOpType.mult)
                tt = acc_live[1]
                nc.tensor.matmul(acc[:], lhsT=mex[:],
                                 rhs=g_c[:, j, 0:D + 1],
                                 start=(tt == 0), stop=(tt == nhalf - 1))
                acc_live[1] = tt + 1
            if not last:
                return
            # y = acc[:, :D] / denom (b added at the finisher)
            recip = sc_p.tile([P, 1], f32, tag="recip")
            nc.vector.reciprocal(recip[:], acc[:, D:D + 1])
            if stage is None:
                stage = fin_p.tile([P, GF * P], f32, tag="stage")
            gi = len(stage_blocks)
            nc.scalar.activation(stage[:, gi * P:gi * P + D], acc[:, 0:D],
                                 mybir.ActivationFunctionType.Copy,
                                 scale=recip[:])
            stage_blocks.append(b)
            if len(stage_blocks) == GF:
                flush_stage()

        # 1-deep software pipeline: half i+1's prologue issues before half
        # i's tile loop, so raw/gathers are ready when the tile loop starts
        prev = stage_a(0)
        for i in range(len(sched)):
            nxt = stage_a(i + 1) if i + 1 < len(sched) else None
            stage_b(i, *prev)
            prev = nxt
        flush_stage()

    nc.compile()
    return nc


def _prep(x, edge_index, W, b, a, a_bias):
    """Host-side sharding/layout. Returns (TL, TH, per-core input maps)."""
    import ml_dtypes

    bf16 = ml_dtypes.bfloat16
    f8 = ml_dtypes.float8_e4m3fn
    x = np.asarray(x, np.float32)
    ei = np.asarray(edge_index)
    W = np.asarray(W, np.float32)
    b = np.asarray(b, np.float32)
    a = np.asarray(a, np.float32)
    a_bias = float(np.asarray(a_bias))

    a_src, a_dst = a[:D], a[D:]
    Wp = np.zeros((P, ELW), np.float32)
    Wp[:, :D] = W
    Wp[:, 129] = W @ a_dst
    Wp[:, 130] = W @ a_src
    C = float(b @ a_dst + b @ a_src + a_bias)
    cb = np.zeros((1, 2 + GF * P), np.float32)
    cb[0, 0] = C
    cb[0, 1] = 0.01 * C
    cb[0, 2:] = np.tile(b, GF)

    sl = np.arange(N, dtype=np.int64)
    src = np.concatenate([ei[0].astype(np.int64), sl])
    dst = np.concatenate([ei[1].astype(np.int64), sl])
    # per-core table rotation: core c stores original node n at table row
    # (n - c*NPC) mod N, so its own src rows are table rows 0..NPC-1.
    # dst indices below are in each core's rotated space.
    core_all = src // NPC
    blk_all = (src - core_all * NPC) // P
    dstr = (dst - core_all * NPC) % N
    # sort by (src block, dst half, dst) so each tile is one block and one
    # half, with ascending dst inside each group (HBM gather locality)
    half_all = (dstr >= SPLIT).astype(np.int64)
    order = np.lexsort((dstr, half_all, core_all * NBLK + blk_all))
    src, dstr, half_all = src[order], dstr[order], half_all[order]
    core_all = src // NPC
    blk_all = (src - core_all * NPC) // P

    # per-(core, block, half) counts -> shared tile schedule
    counts = np.zeros((NCORES, NBLK, 2), np.int64)
    np.add.at(counts, (core_all, blk_all, half_all), 1)
    tl = (counts[:, :, 0] + P - 1) // P
    th = (counts[:, :, 1] + P - 1) // P
    TL = np.maximum(tl.max(axis=0), 1)
    TH = np.maximum(th.max(axis=0), 1)
    T_total = int((TL + TH).sum())

    cstart = np.searchsorted(src, np.arange(NCORES) * NPC)
    cend = np.append(cstart[1:], len(src))
    n_nt = math.ceil(N / P)
    xTf = x.T.astype(bf16)
    Wp = Wp.astype(bf16)

    core_inputs = []
    for c in range(NCORES):
        s, e = cstart[c], cend[c]
        csrc, cdst, chalf = src[s:e], dstr[s:e], half_all[s:e]
        rot = (np.arange(N) + c * NPC) % N
        xT = np.zeros((P, n_nt * P), bf16)
        xT[:, :N] = xTf[:, rot]
        key = (csrc - c * NPC) // P * 2 + chalf
        kstart = np.searchsorted(key, np.arange(NBLK * 2))
        kend = np.append(kstart[1:], len(csrc))
        idx_flat = np.zeros(T_total * P, np.int16)
        src_flat = np.full(T_total * P, PAD_SRC, np.float32)
        t0 = 0
        for bb in range(NBLK):
            for hh, tcnt, base in ((0, int(TL[bb]), 0), (1, int(TH[bb]),
                                                         SPLIT)):
                ks, ke = int(kstart[bb * 2 + hh]), int(kend[bb * 2 + hh])
                nbe = ke - ks
                d_pad = np.zeros(tcnt * P, np.int16)
                s_pad = np.full(tcnt * P, PAD_SRC, np.float32)
                d_pad[:nbe] = (cdst[ks:ke] - base).astype(np.int16)
                s_pad[:nbe] = (csrc[ks:ke] - c * NPC - bb * P).astype(
                    np.float32)
                idx_flat[t0 * P:(t0 + tcnt) * P] = d_pad
                src_flat[t0 * P:(t0 + tcnt) * P] = s_pad
                t0 += tcnt
        # idx wrap: flat i -> [i%16, i//16], replicated down 128 partitions
        idx_w = np.zeros((P, T_total * 8), np.int16)
        wr = idx_flat.reshape(T_total * 8, 16).T
        for rep in range(8):
            idx_w[rep * 16:(rep + 1) * 16] = wr
        src_arr = np.ascontiguousarray(
            src_flat.reshape(T_total, P).T.astype(np.float32))
        # transposed one-hot: eqT8[i, t*128+e] = (src_local[t,e] == i)
        eq_arr = np.zeros((P, T_total * P), f8)
        sf = src_flat.astype(np.int64)
        valid = sf < P
        eq_arr[sf[valid], np.nonzero(valid)[0]] = 1.0
        core_inputs.append({
            "xT": xT,
            "Wp": Wp,
            "cb": cb,
            "idxW": idx_w,
            "srcT": src_arr,
            "eqT8": eq_arr,
        })
    return TL, TH, core_inputs


_last_results = None


def kernel(x, edge_index, W, b, a, a_bias):
    global _last_results
    from concourse.bass_utils import run_bass_kernel_spmd

    TL, TH, core_inputs = _prep(x, edge_index, W, b, a, a_bias)
    key = (tuple(int(v) for v in TL), tuple(int(v) for v in TH))
    if key not in _cache:
        _cache[key] = _build_program(TL, TH)
    nc = _cache[key]
    res = run_bass_kernel_spmd(nc, core_inputs, core_ids=list(range(NCORES)),
                               trace=False)
    _last_results = res
    outs = [res.results[c]["out"] for c in range(NCORES)]
    return np.concatenate(outs, axis=0)


# revision 45
# speedup vs baseline: 1.0813x; 1.0813x over previous
"""GAT forward on 8 Trainium2 NeuronCores (Bass/Tile, SPMD, no collectives).

Sharding: edges assigned to cores by src-node range (N/8 nodes per core);
each core computes the full output rows for its own src range.

Structure (v4): a bf16 node table [N, 256] holds [xW | 1 | v' | u'] per
node (512B stride, b NOT folded in). Edge-tile rows are fetched with
InstDMAGatherAnt (260B payload per edge, int16 indices, so each src
block's edges are split into dst<32768 / dst>=32768 groups gathered from
base-offset views). Per-edge u'[src] is selected from a per-block u
column by a tiny PE matmul against a host-streamed fp8 transposed
one-hot; exp(leaky_relu) is computed on [128, tiles] columns with the
b/a_bias constants folded into the activation bias; the one-hot scatter
matrix is a single fused tensor_scalar(is_equal, mult) per tile feeding
a bf16 aggregation matmul with PSUM accumulation per block. The linear
bias b is added at the batched ELU finisher (sum(alpha)=1 makes this
exact). Softmax max-subtraction is skipped: logits are bounded so exp is
well-conditioned and the result is mathematically identical.

v4 vs v3: phase 1 drops the rank-1 bias matmul (consts -> exp bias,
b -> finisher add), batches 3 matmul outputs per PSUM bank with one
Scalar copy per group, and memsets the "1" column; per-block u-column
indirect gathers are spread through the block loop; pipeline pools are
deepened so SWDGE gathers run several halves ahead.
"""
import math

import numpy as np

N, E, D, P = 50000, 1600000, 128, 128
NCORES = 8
NPC = N // NCORES          # nodes per core
NBLK = math.ceil(NPC / P)  # src blocks per core
STEP = 256                 # table row stride (bf16 elems; 512B)
ELW = 131                  # table row payload written: [xW |1| v' | u']
ELG = 130                  # gathered payload elems: [xW |1| v'] (260B)
PAD_SRC = 999.0
SPLIT = 32768              # int16 index range split
IC = 256                   # tiles per srcT DRAM load
EC = 16                    # tiles per eqT DRAM load
GF = 4                     # blocks per batched ELU finisher
GMAX = 4                   # tiles per dma_gather (512 desc; 2 fit per ring)

_cache = {}


def _dma_gather(gp, out_ap, in_ap, idxs_ap, num_idxs, elem_size, elem_step,
                queue_num=0):
    """dma_gather (non-transpose, HBM source) with the %256 payload assert
    relaxed; the row stride must still be a 256B multiple (HW field)."""
    import concourse.mybir as mybir
    from concourse import ap_utils

    assert idxs_ap.dtype == mybir.dt.int16
    assert in_ap.dtype == out_ap.dtype
    dts = mybir.dt.size(in_ap.dtype)
    assert ap_utils.ap_is_contiguous(out_ap.ap[1:])
    assert ap_utils.ap_is_contiguous(idxs_ap.ap[1:])
    assert num_idxs % 128 == 0
    assert in_ap.ap[-1][1] == elem_size and out_ap.ap[-1][1] == elem_size
    assert out_ap.ap[0][1] * out_ap.ap[1][1] == num_idxs
    stride_bytes = elem_step * dts
    assert stride_bytes % 256 == 0 and stride_bytes // 256 < 256
    assert in_ap.ap[0][0] == elem_step
    _in_ap = gp.lower_ap_dma(in_ap, for_custom_bir_dma=True)
    _idxs_ap = gp.lower_ap(idxs_ap)
    _out_ap = gp.lower_ap(out_ap)
    return gp.add_instruction(
        mybir.InstDMAGatherAnt(
            name=gp.bass.get_next_instruction_name(),
            ins=[*_in_ap, _idxs_ap, gp.lower_val_access(gp.to_reg(num_idxs))],
            outs=[_out_ap],
            transpose=False,
            num_idxs=num_idxs,
            elem_size=elem_size,
            stride_bytes_256=stride_bytes // 256,
            gen_mode=0,
            single_packet=True,
            queue_num=queue_num,
            sbuf_tokens_per_rank=0,
            sbuf_free_dim_per_rank=0,
            sbuf_free_dim_pad_per_rank=0,
            sbuf_byte_offset=0,
        ))


def _build_program(TL, TH):
    """Build the SPMD bass program. TL/TH = per-block tile counts for the
    low/high dst halves (len NBLK each, shared schedule across cores)."""
    from contextlib import ExitStack
    import concourse.bass as bass
    import concourse.bacc as bacc
    import concourse.mybir as mybir
    import concourse.tile as tile

    T_per = [int(TL[b]) + int(TH[b]) for b in range(NBLK)]
    T_total = int(sum(T_per))
    Tmax = int(max(max(TL), max(TH)))
    n_nt = math.ceil(N / P)            # phase-1 node tiles
    N_pad = n_nt * P
    nc = bacc.Bacc("TRN2", target_bir_lowering=False, debug=False,
                   num_swdge_queues=4)

    bf16 = mybir.dt.bfloat16
    f32 = mybir.dt.float32
    i32 = mybir.dt.int32
    i16 = mybir.dt.int16
    f8 = mybir.dt.float8e4

    xT = nc.dram_tensor("xT", [P, N_pad], bf16, kind="ExternalInput")
    Wp = nc.dram_tensor("Wp", [P, ELW], bf16, kind="ExternalInput")
    cb = nc.dram_tensor("cb", [1, 2 + GF * P], f32, kind="ExternalInput")
    idxW = nc.dram_tensor("idxW", [P, T_total * 8], i16, kind="ExternalInput")
    srcT = nc.dram_tensor("srcT", [P, T_total], f32, kind="ExternalInput")
    eqT8 = nc.dram_tensor("eqT8", [P, T_total * P], f8, kind="ExternalInput")
    out = nc.dram_tensor("out", [NPC, D], f32, kind="ExternalOutput")

    with tile.TileContext(nc) as tc, ExitStack() as ctx:
        const_p = ctx.enter_context(tc.tile_pool(name="const", bufs=1))
        dram_p = ctx.enter_context(tc.tile_pool(name="dram", bufs=1, space="DRAM"))
        x_p = ctx.enter_context(tc.tile_pool(name="x", bufs=2))
        h_p = ctx.enter_context(tc.tile_pool(name="h", bufs=3))
        ps1_p = ctx.enter_context(tc.tile_pool(name="ps1", bufs=4, space="PSUM"))
        idx_p = ctx.enter_context(tc.tile_pool(name="idx", bufs=6))
        eq_p = ctx.enter_context(tc.tile_pool(name="eq", bufs=5))
        g_p = ctx.enter_context(tc.tile_pool(name="g", bufs=6))
        sc_p = ctx.enter_context(tc.tile_pool(name="sc", bufs=4))
        ups_p = ctx.enter_context(tc.tile_pool(name="ups", bufs=2, space="PSUM"))
        mex_p = ctx.enter_context(tc.tile_pool(name="mex", bufs=8))
        acc_ps = ctx.enter_context(tc.tile_pool(name="accps", bufs=2, space="PSUM"))
        fin_p = ctx.enter_context(tc.tile_pool(name="fin", bufs=3))

        h_ext = dram_p.tile([N_pad, STEP], bf16)

        # constants
        iota_i = const_p.tile([P, P], i32)
        nc.gpsimd.iota(iota_i[:], [[1, P]], channel_multiplier=0)
        iota_b = const_p.tile([P, P], bf16)
        nc.vector.tensor_copy(iota_b[:], iota_i[:])
        ones_row = const_p.tile([1, P], bf16)
        nc.vector.memset(ones_row[:], 1.0)

        Wp_t = const_p.tile([P, ELW], bf16)
        nc.sync.dma_start(Wp_t[:], Wp[:, :])
        u_cols = const_p.tile([P, NBLK], bf16)
        # broadcast [1, 2+GF*P] consts (C, 0.01*C, b tiled GF times) to all
        # partitions via contraction-1 PE matmuls (PSUM bank caps at 512 f32)
        ones1f = const_p.tile([1, P], f32)
        nc.vector.memset(ones1f[:], 1.0)
        neg1_col = const_p.tile([P, 1], f32)
        nc.vector.memset(neg1_col[:], -1.0)
        cb_row = const_p.tile([1, 2 + GF * P], f32)
        nc.sync.dma_start(cb_row[:], cb[:, :])
        cb_t = const_p.tile([P, 2 + GF * P], f32)
        cbC_ps = ps1_p.tile([P, 2], f32, tag="ps1")
        nc.tensor.matmul(cbC_ps[:], lhsT=ones1f[:], rhs=cb_row[:, 0:2],
                         start=True, stop=True)
        nc.scalar.copy(cb_t[:, 0:2], cbC_ps[:])
        b4_ps = ps1_p.tile([P, GF * P], f32, tag="ps1")
        nc.tensor.matmul(b4_ps[:], lhsT=ones1f[:], rhs=cb_row[:, 2:2 + GF * P],
                         start=True, stop=True)
        nc.scalar.copy(cb_t[:, 2:2 + GF * P], b4_ps[:])
        C_col = cb_t[:, 0:1]
        C2_col = cb_t[:, 1:2]
        b_rep = cb_t[:, 2:2 + GF * P]

        # ---- phase 1: h_ext[n, 0:ELW] = [xW | 1 | v' | u'] (bf16) ----
        XC = 16  # node tiles per x chunk / per staged table write
        for c0 in range(0, n_nt, XC):
            cn = min(XC, n_nt - c0)
            xc = x_p.tile([P, XC * P], bf16, tag="xc")
            nc.scalar.dma_start(xc[:, :cn * P], xT[:, c0 * P:(c0 + cn) * P])
            hs = h_p.tile([P, XC * ELW], bf16, tag="hs")
            for g0 in range(0, cn, 3):
                gn = min(3, cn - g0)
                ps = ps1_p.tile([P, 3 * ELW], f32, tag="ps1")
                for k in range(gn):
                    j = g0 + k
                    nc.tensor.matmul(ps[:, k * ELW:(k + 1) * ELW],
                                     lhsT=xc[:, j * P:(j + 1) * P],
                                     rhs=Wp_t[:], start=True, stop=True)
                nc.vector.tensor_copy(hs[:, g0 * ELW:(g0 + gn) * ELW],
                                      ps[:, :gn * ELW])
            hs_v = hs[:].rearrange("p (j c) -> p j c", c=ELW)
            nc.vector.memset(hs_v[:, :cn, 128:129], 1.0)
            # the table is rotated per-core so this core's src blocks are
            # tiles 0..NBLK-1: peel their u' columns off the staging tile
            for j in range(cn):
                gidx = c0 + j
                if gidx < NBLK:
                    nc.vector.tensor_copy(
                        u_cols[:, gidx:gidx + 1],
                        hs[:, j * ELW + 130:j * ELW + 131])
            dst = h_ext[c0 * P:(c0 + cn) * P, 0:ELW]
            nc.scalar.dma_start(
                dst.rearrange("(j p) c -> p j c", p=P),
                hs[:, :cn * ELW].rearrange("p (j c) -> p j c", c=ELW))

        # ---- phase 2 ----
        src_tiles = {}

        def get_src(t):
            c0 = (t // IC) * IC
            if c0 not in src_tiles:
                cn = min(IC, T_total - c0)
                s_c = idx_p.tile([P, IC], f32, tag="srcc")
                nc.sync.dma_start(s_c[:, :cn], srcT[:, c0:c0 + cn])
                src_tiles[c0] = s_c
            return src_tiles[c0], t - c0

        eq_tiles = {}

        def get_eq(t):
            c0 = (t // EC) * EC
            if c0 not in eq_tiles:
                cn = min(EC, T_total - c0)
                e_c = eq_p.tile([P, EC * P], f8, tag="eqc")
                nc.sync.dma_start(e_c[:, :cn * P],
                                  eqT8[:, c0 * P:(c0 + cn) * P])
                eq_tiles[c0] = e_c
            return eq_tiles[c0], t - c0

        # batched ELU finisher staging
        stage = None
        stage_blocks = []

        def flush_stage():
            nonlocal stage, stage_blocks
            if not stage_blocks:
                return
            w = len(stage_blocks) * P
            yb = fin_p.tile([P, GF * P], f32, tag="yb")
            nc.vector.tensor_tensor(out=yb[:, :w], in0=stage[:, :w],
                                    in1=b_rep[:, :w], op=mybir.AluOpType.add)
            mp = fin_p.tile([P, GF * P], f32, tag="mp")
            nc.vector.tensor_scalar(
                out=mp[:, :w], in0=yb[:, :w], scalar1=0.0, scalar2=1.0,
                op0=mybir.AluOpType.min, op1=mybir.AluOpType.add)
            ex = fin_p.tile([P, GF * P], f32, tag="ex")
            nc.scalar.activation(ex[:, :w], mp[:, :w],
                                 mybir.ActivationFunctionType.Exp,
                                 bias=neg1_col[:])
            tm = fin_p.tile([P, GF * P], f32, tag="tm")
            nc.vector.tensor_tensor(out=tm[:, :w], in0=yb[:, :w],
                                    in1=mp[:, :w], op=mybir.AluOpType.subtract)
            ot = fin_p.tile([P, GF * P], f32, tag="ot")
            nc.vector.tensor_tensor(out=ot[:, :w], in0=tm[:, :w],
                                    in1=ex[:, :w], op=mybir.AluOpType.add)
            for gi, b in enumerate(stage_blocks):
                rows_b = min(P, NPC - b * P)
                nc.sync.dma_start(out[b * P:b * P + rows_b, :],
                                  ot[:rows_b, gi * P:gi * P + D])
            stage = None
            stage_blocks = []

        # flat half schedule: (block, th, base, t0, first, last, nhalf)
        sched = []
        t = 0
        for b in range(NBLK):
            nhalf = int(TL[b]) + int(TH[b])
            halves = [(th, base) for th, base in
                      ((int(TL[b]), 0), (int(TH[b]), SPLIT)) if th]
            for hi, (th, base) in enumerate(halves):
                sched.append((b, th, base, t, hi == 0,
                              hi == len(halves) - 1, nhalf))
                t += th

        next_q = [0]

        def stage_a(i):
            """Prologue for half i: idx load, gathers, u' select, raw."""
            b, th, base, t0, _, _, _ = sched[i]
            ix = idx_p.tile([P, Tmax * 8], i16, tag="ix")
            nc.sync.dma_start(ix[:, :th * 8], idxW[:, t0 * 8:(t0 + th) * 8])
            g_c = g_p.tile([P, Tmax, ELG], bf16, tag="g")
            # ucode SWDGE ring caps 1024 descriptors; round-robin 4 queues
            for k0 in range(0, th, GMAX):
                gn = min(GMAX, th - k0)
                _dma_gather(nc.gpsimd, g_c[:, k0:k0 + gn, :],
                            h_ext[base:, 0:ELG],
                            ix[:, k0 * 8:(k0 + gn) * 8],
                            gn * P, ELG, STEP,
                            queue_num=next_q[0])
                next_q[0] = (next_q[0] + 1) % 4
            # u'[src_e] columns via fp8 one-hot matmuls
            u_ps = ups_p.tile([P, Tmax], f32, tag="ups")
            for j in range(th):
                e_c, j_loc = get_eq(t0 + j)
                nc.tensor.matmul(
                    u_ps[:, j:j + 1],
                    lhsT=e_c[:, j_loc * P:(j_loc + 1) * P],
                    rhs=u_cols[:, b:b + 1], start=True, stop=True)
            # s = u' + v';  raw = max(exp(s + C), exp(.01 (s + C)))
            s_c = sc_p.tile([P, Tmax], f32, tag="s")
            nc.vector.tensor_tensor(
                out=s_c[:, :th], in0=u_ps[:, :th],
                in1=g_c[:, :th, 129:130], op=mybir.AluOpType.add)
            e1 = sc_p.tile([P, Tmax], f32, tag="e1")
            nc.scalar.activation(e1[:, :th], s_c[:, :th],
                                 mybir.ActivationFunctionType.Exp,
                                 bias=C_col)
            e2 = sc_p.tile([P, Tmax], f32, tag="e2")
            nc.scalar.activation(e2[:, :th], s_c[:, :th],
                                 mybir.ActivationFunctionType.Exp,
                                 scale=0.01, bias=C2_col)
            raw = sc_p.tile([P, Tmax], f32, tag="raw")
            nc.vector.tensor_tensor(out=raw[:, :th], in0=e1[:, :th],
                                    in1=e2[:, :th], op=mybir.AluOpType.max)
            return g_c, raw

        acc_live = [None, 0]

        def stage_b(i, g_c, raw):
            """Tile loop for half i: one-hot scatter matmuls + finisher."""
            nonlocal stage, stage_blocks
            b, th, base, t0, first, last, nhalf = sched[i]
            if first:
                acc = acc_ps.tile([P, D + 1], f32, tag="acc")
                acc_live[0] = acc
                acc_live[1] = 0
            acc = acc_live[0]
            for j in range(th):
                s_t, s_loc = get_src(t0 + j)
                mex = mex_p.tile([P, P], bf16, tag="mex")
                nc.vector.tensor_scalar(
                    out=mex[:], in0=iota_b[:],
                    scalar1=s_t[:, s_loc:s_loc + 1],
                    scalar2=raw[:, j:j + 1],
                    op0=mybir.AluOpType.is_equal,
                    op1=mybir.AluOpType.mult)
                tt = acc_live[1]
                nc.tensor.matmul(acc[:], lhsT=mex[:],
                                 rhs=g_c[:, j, 0:D + 1],
                                 start=(tt == 0), stop=(tt == nhalf - 1))
                acc_live[1] = tt + 1
            if not last:
                return
            # y = acc[:, :D] / denom (b added at the finisher)
            recip = sc_p.tile([P, 1], f32, tag="recip")
            nc.vector.reciprocal(recip[:], acc[:, D:D + 1])
            if stage is None:
                stage = fin_p.tile([P, GF * P], f32, tag="stage")
            gi = len(stage_blocks)
            nc.scalar.activation(stage[:, gi * P:gi * P + D], acc[:, 0:D],
                                 mybir.ActivationFunctionType.Copy,
                                 scale=recip[:])
            stage_blocks.append(b)
            if len(stage_blocks) == GF:
                flush_stage()

        # 1-deep software pipeline: half i+1's prologue issues before half
        # i's tile loop, so raw/gathers are ready when the tile loop starts
        prev = stage_a(0)
        for i in range(len(sched)):
            nxt = stage_a(i + 1) if i + 1 < len(sched) else None
            stage_b(i, *prev)
            prev = nxt
        flush_stage()

    nc.compile()
    return nc


def _prep(x, edge_index, W, b, a, a_bias):
    """Host-side sharding/layout. Returns (TL, TH, per-core input maps)."""
    import ml_dtypes

    bf16 = ml_dtypes.bfloat16
    f8 = ml_dtypes.float8_e4m3fn
    x = np.asarray(x, np.float32)
    ei = np.asarray(edge_index)
    W = np.asarray(W, np.float32)
    b = np.asarray(b, np.float32)
    a = np.asarray(a, np.float32)
    a_bias = float(np.asarray(a_bias))

    a_src, a_dst = a[:D], a[D:]
    Wp = np.zeros((P, ELW), np.float32)
    Wp[:, :D] = W
    Wp[:, 129] = W @ a_dst
    Wp[:, 130] = W @ a_src
    C = float(b @ a_dst + b @ a_src + a_bias)
    cb = np.zeros((1, 2 + GF * P), np.float32)
    cb[0, 0] = C
    cb[0, 1] = 0.01 * C
    cb[0, 2:] = np.tile(b, GF)

    sl = np.arange(N, dtype=np.int64)
    src = np.concatenate([ei[0].astype(np.int64), sl])
    dst = np.concatenate([ei[1].astype(np.int64), sl])
    # per-core table rotation: core c stores original node n at table row
    # (n - c*NPC) mod N, so its own src rows are table rows 0..NPC-1.
    # dst indices below are in each core's rotated space.
    core_all = src // NPC
    blk_all = (src - core_all * NPC) // P
    dstr = (dst - core_all * NPC) % N
    # sort by (src block, dst half, dst) so each tile is one block and one
    # half, with ascending dst inside each group (HBM gather locality)
    half_all = (dstr >= SPLIT).astype(np.int64)
    order = np.lexsort((dstr, half_all, core_all * NBLK + blk_all))
    src, dstr, half_all = src[order], dstr[order], half_all[order]
    core_all = src // NPC
    blk_all = (src - core_all * NPC) // P

    # per-(core, block, half) counts -> shared tile schedule
    counts = np.zeros((NCORES, NBLK, 2), np.int64)
    np.add.at(counts, (core_all, blk_all, half_all), 1)
    tl = (counts[:, :, 0] + P - 1) // P
    th = (counts[:, :, 1] + P - 1) // P
    TL = np.maximum(tl.max(axis=0), 1)
    TH = np.maximum(th.max(axis=0), 1)
    T_total = int((TL + TH).sum())

    cstart = np.searchsorted(src, np.arange(NCORES) * NPC)
    cend = np.append(cstart[1:], len(src))
    n_nt = math.ceil(N / P)
    xTf = x.T.astype(bf16)
    Wp = Wp.astype(bf16)

    core_inputs = []
    for c in range(NCORES):
        s, e = cstart[c], cend[c]
        csrc, cdst, chalf = src[s:e], dstr[s:e], half_all[s:e]
        rot = (np.arange(N) + c * NPC) % N
        xT = np.zeros((P, n_nt * P), bf16)
        xT[:, :N] = xTf[:, rot]
        key = (csrc - c * NPC) // P * 2 + chalf
        kstart = np.searchsorted(key, np.arange(NBLK * 2))
        kend = np.append(kstart[1:], len(csrc))
        idx_flat = np.zeros(T_total * P, np.int16)
        src_flat = np.full(T_total * P, PAD_SRC, np.float32)
        t0 = 0
        for bb in range(NBLK):
            for hh, tcnt, base in ((0, int(TL[bb]), 0), (1, int(TH[bb]),
                                                         SPLIT)):
                ks, ke = int(kstart[bb * 2 + hh]), int(kend[bb * 2 + hh])
                nbe = ke - ks
                d_pad = np.zeros(tcnt * P, np.int16)
                s_pad = np.full(tcnt * P, PAD_SRC, np.float32)
                d_pad[:nbe] = (cdst[ks:ke] - base).astype(np.int16)
                s_pad[:nbe] = (csrc[ks:ke] - c * NPC - bb * P).astype(
                    np.float32)
                idx_flat[t0 * P:(t0 + tcnt) * P] = d_pad
                src_flat[t0 * P:(t0 + tcnt) * P] = s_pad
                t0 += tcnt
        # idx wrap: flat i -> [i%16, i//16], replicated down 128 partitions
        idx_w = np.zeros((P, T_total * 8), np.int16)
        wr = idx_flat.reshape(T_total * 8, 16).T
        for rep in range(8):
            idx_w[rep * 16:(rep + 1) * 16] = wr
        src_arr = np.ascontiguousarray(
            src_flat.reshape(T_total, P).T.astype(np.float32))
        # transposed one-hot: eqT8[i, t*128+e] = (src_local[t,e] == i)
        eq_arr = np.zeros((P, T_total * P), f8)
        sf = src_flat.astype(np.int64)
        valid = sf < P
        eq_arr[sf[valid], np.nonzero(valid)[0]] = 1.0
        core_inputs.append({
            "xT": xT,
            "Wp": Wp,
            "cb": cb,
            "idxW": idx_w,
            "srcT": src_arr,
            "eqT8": eq_arr,
        })
    return TL, TH, core_inputs


_last_results = None


def kernel(x, edge_index, W, b, a, a_bias):
    global _last_results
    from concourse.bass_utils import run_bass_kernel_spmd

    TL, TH, core_inputs = _prep(x, edge_index, W, b, a, a_bias)
    key = (tuple(int(v) for v in TL), tuple(int(v) for v in TH))
    if key not in _cache:
        _cache[key] = _build_program(TL, TH)
    nc = _cache[key]
    res = run_bass_kernel_spmd(nc, core_inputs, core_ids=list(range(NCORES)),
                               trace=False)
    _last_results = res
    outs = [res.results[c]["out"] for c in range(NCORES)]
    return np.concatenate(outs, axis=0)
